# revision 1
# baseline (speedup 1.0000x reference)
"""Memory-augmented attention kernel for Trainium2 (Bass/Tile), 8-core data parallel.

Reference computation (per row b of B=32768, D=512, K=5):
    q' = query@Wq + bq
    k  = mem@Wk + bk ; v = mem@Wv + bv
    scores = (q'.k_j)/sqrt(D) masked-softmax -> w
    mem_out = (sum_j w_j v_j)@Wo + bo
    gate = sigmoid([query, mem_out]@Wg + bg); conf = sigmoid(max_sim - 0.7)
    out = LN(query + gate*conf*mem_out) * ln_g + ln_b

Algebraic refactoring (all biases are zero and LN affine is identity in this
problem; a numpy fallback covers the general case):
    scores_bk = m_bk . (query_b @ (Wq @ Wk^T)) * scale
    mem_out_b = (sum_k w_bk m_bk) @ (Wv @ Wo)
    gate_b    = sigmoid(query_b . Wg[:D] + mcomb_b . (Wv@Wo@Wg[D:]))

Device mapping per 128-row tile (4-stage software pipeline, lag 3, so each
engine's in-order stream interleaves work from adjacent tiles):
    PE   : transpose q and mcomb (bf16), t = q@Wqk, mem = mcomb@Wvo, gate dots
           (all matmuls bf16 with fp32 PSUM accumulate; 1/sqrt(D) folded into
           Wqk on the host)
    DVE  : scores dot-products and the w-weighted memory combine via native
           scalar_tensor_tensor with accum_out (fp32), softmax glue, fused
           (mem*s)+q with free row-sum, LN scalar glue
    ACT  : exp / ln (rstd = exp(-0.5 ln(var+eps))), sigmoids via exp,
           PSUM->SBUF copies with bf16 casts, Square-acc for E[x^2], final LN
           apply. Only {Copy,Identity,Exp,Ln,Square} are used - one activation
           table, no table reloads.
    GPSIMD: q bf16 cast, mask penalty add, out-DMA via SWDGE

This container's walrus build only encodes one sync-wait per instruction and
cannot encode TENSOR_TENSOR_REDUCE / EVENT_SEMAPHORE_RANGE_CLEAR /
Pool-engine TensorScalarPtr; see _install_tile_patches and the single-dep
"touch" absorber ops below.
"""

import numpy as np

B, D, K = 32768, 512, 5
N_CORES = 8
ROWS = B // N_CORES        # rows per core
P = 128                    # partitions
NT_FULL = ROWS // P        # tiles per core (32)
NCH = D // P               # 128-contraction chunks (4)
SCALE = float(D) ** -0.5
BIG = 1.0e30
LN_EPS = 1e-5
SIM_THRESH = 0.7

_CACHE = {}

TRACE = False              # set by test harness to collect a HW profile
LAST_RESULTS = None        # BassKernelResults of the last run (for profiling)



def _install_tile_patches():
    """Work around two walrus limitations in this container:
    - instructions accept very few sync-wait slots: split the kernel-tail
      drain (which Tile loads with one wait per outstanding semaphore) into
      a chain of single-wait drains;
    - EVENT_SEMAPHORE_RANGE_CLEAR is not encodable: skip the on-device sem
      clear (each kernel() call executes a freshly loaded NEFF) while keeping
      the allocator bookkeeping.
    """
    import concourse.tile as tile
    from concourse.vector_clock import ScopedClock

    if getattr(tile.TileContext._drain_and_barrier, "_patched", False):
        return

    def patched(self, tick_clock, wait_clock):
        import bass_rust

        nc = self.nc
        drain_inst = nc.sync.drain()
        wait_clock.add_sem_waits(
            drain_inst.ins, ScopedClock({None: tick_clock.global_clock})
        )
        si = drain_inst.ins.sync_info
        waits = list(si.on_wait) if si is not None and si.on_wait else []
        if len(waits) > 1:
            drain_inst.ins.sync_info = bass_rust.SyncInfo(
                on_wait=waits[:1], on_update=list(si.on_update or [])
            )
            for w in waits[1:]:
                d2 = nc.sync.drain()
                d2.ins.sync_info = bass_rust.SyncInfo(on_wait=[w], on_update=[])
        nc.all_engine_barrier()
        assert self.sems is not None
        popped = nc._tile_sem_poison_stack.pop()
        assert popped is self._sem_poison
        sems = list(self.sems.allocated().values())
        sem_nums = [s.num for s in sems]
        nc._state.prepend_free_semaphores(sem_nums)
        for poison_set in nc._tile_sem_poison_stack:
            poison_set.update(sem_nums)
        nc.all_engine_barrier()

    patched._patched = True
    tile.TileContext._drain_and_barrier = patched

    # This walrus build accepts at most one sync-wait per instruction:
    # at commit time, peel off extra waits onto single-wait drain
    # instructions inserted just before the owner.
    _orig_commit = tile.TileContext._commit_instruction

    def commit_patched(self, inst, lazy_reg_writes=True):
        import bass_rust
        from concourse import mybir

        si = inst.sync_info
        if si is not None and si.on_wait and len(si.on_wait) > 1:
            waits = list(si.on_wait)
            inst.sync_info = bass_rust.SyncInfo(
                on_wait=waits[-1:], on_update=list(si.on_update or [])
            )
            for w in waits[:-1]:
                eng = self.nc.engines[inst.engine]
                if not hasattr(eng, "engine_nop"):
                    nop = mybir.InstDrain(
                        name=self.nc.get_next_instruction_name(), ins=[], outs=[]
                    )
                    nop.engine = inst.engine
                else:
                    # sequencer-only ENGINE_NOP: carries the wait without
                    # flushing the compute pipeline the way a drain does
                    nop = eng.engine_nop().ins
                nop.sync_info = bass_rust.SyncInfo(on_wait=[w], on_update=[])
                self._add_instruction(nop)
        return _orig_commit(self, inst, lazy_reg_writes)

    tile.TileContext._commit_instruction = commit_patched


def _build(ntiles=NT_FULL):
    import concourse.bass as bass
    import concourse.tile as tile
    from concourse import mybir

    _install_tile_patches()

    f32 = mybir.dt.float32
    f32r = mybir.dt.float32r
    bf16 = mybir.dt.bfloat16
    u8 = mybir.dt.uint8
    AF = mybir.ActivationFunctionType
    OP = mybir.AluOpType
    AX = mybir.AxisListType

    rows = ntiles * P
    rD = 1.0 / float(D)

    nc = bass.Bass()
    qm_d = nc.declare_dram_parameter("qm", [rows, (K + 1) * D], f32r, isOutput=False)
    sims_d = nc.declare_dram_parameter("sims", [rows, K], f32, isOutput=False)
    mask_d = nc.declare_dram_parameter("mask", [rows, K], u8, isOutput=False)
    wqk_d = nc.declare_dram_parameter("wqk", [D, D], bf16, isOutput=False)
    wvo_d = nc.declare_dram_parameter("wvo", [D, D], bf16, isOutput=False)
    gv_d = nc.declare_dram_parameter("gv", [D, 2], bf16, isOutput=False)
    id_d = nc.declare_dram_parameter("ident", [P, P], bf16, isOutput=False)
    idr_d = nc.declare_dram_parameter("identr", [P, P], f32, isOutput=False)
    o_d = nc.declare_dram_parameter("o", [rows, D], f32, isOutput=True)

    qm_t = qm_d.rearrange("(t p) d -> t p d", p=P)
    o_t = o_d.rearrange("(t p) d -> t p d", p=P)

    with tile.TileContext(nc) as tc:
        with (
            tc.tile_pool(name="consts", bufs=1) as consts,
            tc.tile_pool(name="qmload", bufs=6) as qmload,
            tc.tile_pool(name="work", bufs=3) as work,
            tc.tile_pool(name="smalls", bufs=6) as smalls,
            tc.tile_pool(name="pbig", bufs=5, space="PSUM") as pbig,
            tc.tile_pool(name="pmix", bufs=3, space="PSUM") as pmix,
        ):
            # ---- constants, loaded once ----
            wqk_sb = consts.tile([P, NCH, D], bf16)
            nc.sync.dma_start(out=wqk_sb, in_=wqk_d.rearrange("(c p) e -> p c e", p=P))
            wvo_sb = consts.tile([P, NCH, D], bf16)
            nc.sync.dma_start(out=wvo_sb, in_=wvo_d.rearrange("(c p) e -> p c e", p=P))
            g_sb = consts.tile([P, NCH, 2], bf16)
            nc.sync.dma_start(out=g_sb, in_=gv_d.rearrange("(c p) j -> p c j", p=P))
            ident = consts.tile([P, P], bf16)
            nc.sync.dma_start(out=ident, in_=id_d[:, :])
            identr = consts.tile([P, P], f32)
            nc.sync.dma_start(out=identr, in_=idr_d[:, :])

            sims_all = consts.tile([P, ntiles, K], f32)
            nc.sync.dma_start(
                out=sims_all, in_=sims_d.rearrange("(t p) k -> p t k", p=P)
            )
            mask_all = consts.tile([P, ntiles, K], u8)
            nc.sync.dma_start(
                out=mask_all, in_=mask_d.rearrange("(t p) k -> p t k", p=P)
            )

            thresh = consts.tile([P, 1], f32)
            nc.vector.memset(thresh, SIM_THRESH)
            epsc = consts.tile([P, 1], f32)
            nc.vector.memset(epsc, LN_EPS)

            # conf[b, t] = sigmoid(max_k sims - th) = 1/(1+exp(th - max))
            simmax = consts.tile([P, ntiles], f32)
            nc.vector.reduce_max(out=simmax, in_=sims_all, axis=AX.X)
            confe = consts.tile([P, ntiles], f32)
            nc.scalar.activation(
                out=confe, in_=simmax, func=AF.Exp, bias=thresh, scale=-1.0
            )
            confe1 = consts.tile([P, ntiles], f32)
            nc.vector.tensor_scalar(
                out=confe1, in0=confe, scalar1=1.0, scalar2=None, op0=OP.add
            )
            conf_all = consts.tile([P, ntiles], f32)
            nc.vector.reciprocal(out=conf_all, in_=confe1)

            # pen[b, t, k] = 0 if valid else -BIG
            m01 = consts.tile([P, ntiles, K], f32)
            nc.vector.tensor_copy(out=m01, in_=mask_all)
            pen_all = consts.tile([P, ntiles, K], f32)
            nc.vector.tensor_scalar(
                out=pen_all, in0=m01, scalar1=1.0, scalar2=BIG,
                op0=OP.subtract, op1=OP.mult,
            )

            actabs = consts.tile([P, 2], f32)
            nc.vector.memset(actabs, 0.0)

            def touch_dve(ap):
                tt = smalls.tile([P, 2], f32, tag="dvet", name="dvet")
                nc.vector.tensor_copy(out=tt[:, 0:ap.free_size()], in_=ap)

            def touch_gp(ap):
                tt = smalls.tile([P, 2], f32, tag="gpt", name="gpt")
                nc.gpsimd.tensor_copy(out=tt[:, 0:ap.free_size()], in_=ap)

            def touch_act(ap):
                tt = smalls.tile([P, 2], f32, tag="actt", name="actt")
                nc.scalar.copy(out=tt[:, 0:ap.free_size()], in_=ap)

            # Per-tile live state, keyed by tile index. Three-stage software
            # pipeline (lag 2) so each engine's in-order stream interleaves
            # work from adjacent tiles instead of idling through each tile's
            # serial dependency chain.
            st = {}

            def dma_in(t):
                s = st.setdefault(t, {})
                qm = qmload.tile([P, (K + 1) * D], f32r, tag="qm", name="qmtile")
                nc.sync.dma_start(out=qm, in_=qm_t[t])
                s["qmr"] = qm
                s["q"] = qm[:, 0:D].bitcast(f32)
                s["m"] = qm[:, D:].bitcast(f32)

            def stage_a(t):
                # qT via PE transpose (bf16); t = q@Wqk ; nqdot = -(q.g1)
                s = st[t]
                q_bf = work.tile([P, D], bf16, tag="q_bf")
                touch_gp(s["q"][:, 0:2])
                nc.gpsimd.tensor_copy(out=q_bf, in_=s["q"])
                psum_q = pmix.tile([P, D], bf16, tag="pmix")
                for c in range(NCH):
                    sl = slice(c * P, (c + 1) * P)
                    nc.tensor.transpose(psum_q[:, sl], q_bf[:, sl], ident)
                qT = work.tile([P, D], bf16, tag="qT")
                nc.scalar.copy(out=qT, in_=psum_q)

                s["pt"] = pbig.tile([P, D], f32, tag="pbig", name="pt")
                psum_qg = pmix.tile([P, 1], f32, tag="pmix")
                for c in range(NCH):
                    sl = slice(c * P, (c + 1) * P)
                    nc.tensor.matmul(
                        s["pt"],
                        lhsT=qT[:, sl],
                        rhs=wqk_sb[:, c, :],
                        start=(c == 0), stop=(c == NCH - 1),
                    )
                for c in range(NCH):
                    sl = slice(c * P, (c + 1) * P)
                    nc.tensor.matmul(
                        psum_qg,
                        lhsT=qT[:, sl],
                        rhs=g_sb[:, c, 0:1],
                        start=(c == 0), stop=(c == NCH - 1),
                    )
                s["nqdot"] = smalls.tile([P, 1], f32, tag="nqdot", name="nqdot")
                nc.scalar.activation(
                    out=s["nqdot"], in_=psum_qg, func=AF.Copy, scale=-1.0
                )

            def stage_b(t):
                # scores_k = pen_k + (m_k . t)   (1/sqrt(D) folded into Wqk)
                s = st[t]
                raw = smalls.tile([P, K], f32, tag="rawsc", name="rawsc")
                scratch = work.tile([P, D], f32, tag="scratch")
                touch_dve(s["m"][:, 0:2])
                touch_dve(s["pt"][:, 0:2])
                for k in range(K):
                    nc.vector.scalar_tensor_tensor(
                        out=scratch,
                        in0=s["m"][:, k * D:(k + 1) * D],
                        scalar=1.0,
                        in1=s["pt"],
                        op0=OP.mult, op1=OP.mult,
                        accum_out=raw[:, k:k + 1],
                    )
                s["scores"] = smalls.tile([P, K], f32, tag="scores", name="scores")
                nc.gpsimd.tensor_tensor(
                    out=s["scores"], in0=raw, in1=pen_all[:, t, :], op=OP.add
                )
                s["negrmax"] = smalls.tile([P, 1], f32, tag="negrmax", name="negrmax")
                nc.vector.reduce_max(
                    out=s["negrmax"], in_=s["scores"], axis=AX.X, negate=True
                )

            def stage_c1(t):
                # w = exp(scores - max); unnormalized mcomb' = sum_k w_k m_k;
                # mem' = mcomb'@Wvo ; mdot' = mcomb'.g2 ; rsum = 1/sumexp
                s = st[t]
                s["w"] = smalls.tile([P, K], f32, tag="w", name="wtile")
                sumexp = smalls.tile([P, 1], f32, tag="sumexp", name="sumexp")
                touch_act(s["scores"][:, 0:2])
                nc.scalar.activation(
                    out=s["w"], in_=s["scores"], func=AF.Exp,
                    bias=s["negrmax"], scale=1.0, accum_out=sumexp,
                )
                s["rsum"] = smalls.tile([P, 1], f32, tag="rsum", name="rsum")
                nc.vector.reciprocal(out=s["rsum"], in_=sumexp)
                s["negrsum"] = smalls.tile([P, 1], f32, tag="negrsum", name="negrsum")
                nc.vector.tensor_scalar(
                    out=s["negrsum"], in0=s["rsum"], scalar1=-1.0,
                    scalar2=None, op0=OP.mult,
                )
                # mcomb = sum_k w_k m_k  via diag(w_k) matmuls (fp32r PE)
                touch_dve(s["w"][:, 0:2])
                psum_mc = pbig.tile([P, D], f32, tag="pbig")
                for k in range(K):
                    dk = smalls.tile([P, P], f32r, tag="diag", name="diag")
                    nc.vector.tensor_scalar(
                        out=dk, in0=identr, scalar1=s["w"][:, k:k + 1],
                        scalar2=None, op0=OP.mult,
                    )
                    nc.tensor.matmul(
                        psum_mc,
                        lhsT=dk,
                        rhs=s["qmr"][:, (k + 1) * D:(k + 2) * D],
                        start=(k == 0), stop=(k == K - 1),
                    )
                mcomb_bf = work.tile([P, D], bf16, tag="mcomb_bf")
                touch_act(psum_mc[:, 0:2])
                nc.scalar.copy(out=mcomb_bf, in_=psum_mc)

                psum_mt = pmix.tile([P, D], bf16, tag="pmix")
                for c in range(NCH):
                    sl = slice(c * P, (c + 1) * P)
                    nc.tensor.transpose(psum_mt[:, sl], mcomb_bf[:, sl], ident)
                mcT = work.tile([P, D], bf16, tag="mcT")
                nc.scalar.copy(out=mcT, in_=psum_mt)

                s["pmem"] = pbig.tile([P, D], f32, tag="pbig", name="pmem")
                psum_mg = pmix.tile([P, 1], f32, tag="pmix")
                for c in range(NCH):
                    sl = slice(c * P, (c + 1) * P)
                    nc.tensor.matmul(
                        s["pmem"],
                        lhsT=mcT[:, sl],
                        rhs=wvo_sb[:, c, :],
                        start=(c == 0), stop=(c == NCH - 1),
                    )
                for c in range(NCH):
                    sl = slice(c * P, (c + 1) * P)
                    nc.tensor.matmul(
                        psum_mg,
                        lhsT=mcT[:, sl],
                        rhs=g_sb[:, c, 1:2],
                        start=(c == 0), stop=(c == NCH - 1),
                    )
                s["mdot"] = smalls.tile([P, 1], f32, tag="mdot", name="mdot")
                nc.scalar.copy(out=s["mdot"], in_=psum_mg)

            def stage_c2(t):
                # s = conf*rsum/(1+exp(-(qdot + rsum*mdot'))) ;
                # out_pre = s*mem' + q ; layernorm ; store
                s = st.pop(t)
                touch_act(s["negrsum"][:, 0:1])
                ge = smalls.tile([P, 1], f32, tag="ge")
                nc.scalar.activation(
                    out=ge, in_=s["mdot"], func=AF.Exp,
                    bias=s["nqdot"], scale=s["negrsum"],
                )
                gp1 = smalls.tile([P, 1], f32, tag="gp1")
                nc.vector.tensor_scalar(
                    out=gp1, in0=ge, scalar1=1.0, scalar2=None, op0=OP.add
                )
                rgp = smalls.tile([P, 1], f32, tag="rgp")
                nc.vector.reciprocal(out=rgp, in_=gp1)
                s_sb = smalls.tile([P, 1], f32, tag="s")
                nc.vector.tensor_scalar(
                    out=s_sb, in0=rgp, scalar1=conf_all[:, t:t + 1],
                    scalar2=s["rsum"], op0=OP.mult, op1=OP.mult,
                )

                touch_dve(s["pmem"][:, 0:2])
                touch_dve(s_sb[:, 0:1])
                out_pre = work.tile([P, D], f32, tag="out_pre")
                rowsum = smalls.tile([P, 1], f32, tag="rowsum")
                nc.vector.scalar_tensor_tensor(
                    out=out_pre, in0=s["pmem"], scalar=s_sb, in1=s["q"],
                    op0=OP.mult, op1=OP.add, accum_out=rowsum,
                )

                sumsq = smalls.tile([P, 1], f32, tag="sumsq")
                sqscr = work.tile([P, D], f32, tag="sqscr")
                nc.scalar.activation(
                    out=sqscr, in_=out_pre, func=AF.Square, accum_out=sumsq
                )
                mu = smalls.tile([P, 1], f32, tag="mu")
                nc.vector.tensor_scalar(
                    out=mu, in0=rowsum, scalar1=rD, scalar2=None, op0=OP.mult
                )
                mu2 = smalls.tile([P, 1], f32, tag="mu2")
                nc.gpsimd.tensor_tensor(out=mu2, in0=mu, in1=mu, op=OP.mult)
                varc = smalls.tile([P, 1], f32, tag="varc")
                nc.vector.scalar_tensor_tensor(
                    out=varc, in0=sumsq, scalar=rD, in1=mu2,
                    op0=OP.mult, op1=OP.subtract,
                )
                lnv = smalls.tile([P, 1], f32, tag="lnv")
                nc.scalar.activation(
                    out=lnv, in_=varc, func=AF.Ln, bias=epsc, scale=1.0
                )
                rstd = smalls.tile([P, 1], f32, tag="rstd")
                nc.scalar.activation(out=rstd, in_=lnv, func=AF.Exp, scale=-0.5)
                nmr = smalls.tile([P, 1], f32, tag="nmr")
                nc.vector.tensor_scalar(
                    out=nmr, in0=mu, scalar1=rstd, scalar2=-1.0,
                    op0=OP.mult, op1=OP.mult,
                )
                out_sb = work.tile([P, D], f32, tag="out_sb")
                touch_act(nmr[:, 0:1])
                nc.scalar.memzero(out_sb[:, 0:2])
                nc.scalar.activation(
                    out=out_sb, in_=out_pre, func=AF.Identity, scale=rstd, bias=nmr
                )
                nc.gpsimd.dma_start(out=o_t[t], in_=out_sb)

            dma_in(0)
            for i in range(ntiles + 3):
                if i + 1 < ntiles:
                    dma_in(i + 1)
                if i < ntiles:
                    stage_a(i)
                if 0 <= i - 3:
                    stage_c2(i - 3)
                if 0 <= i - 2 <= ntiles - 1:
                    stage_c1(i - 2)
                if 0 <= i - 1 <= ntiles - 1:
                    stage_b(i - 1)

    return nc


def _numpy_fallback(query, retrieved_memories, similarities, mask,
                    Wq, bq, Wk, bk, Wv, bv, Wo, bo, Wg, bg, ln_g, ln_b):
    x = query.astype(np.float64)
    m = retrieved_memories.astype(np.float64)
    q = x @ Wq + bq
    k = np.einsum("bkd,de->bke", m, Wk.astype(np.float64)) + bk
    v = np.einsum("bkd,de->bke", m, Wv.astype(np.float64)) + bv
    scores = np.einsum("bd,bkd->bk", q, k) * (D ** -0.5)
    scores = np.where(mask, scores, -np.inf)
    sm = scores - scores.max(-1, keepdims=True)
    w = np.exp(sm)
    w /= w.sum(-1, keepdims=True)
    w = np.where(mask, w, 0.0)
    mem = np.einsum("bk,bkd->bd", w, v) @ Wo + bo
    gate = 1 / (1 + np.exp(-(np.concatenate([x, mem], -1) @ Wg + bg)))
    conf = 1 / (1 + np.exp(-(similarities.max(-1, keepdims=True) - SIM_THRESH)))
    out = x + (gate * conf) * mem
    mu = out.mean(-1, keepdims=True)
    var = ((out - mu) ** 2).mean(-1, keepdims=True)
    out = (out - mu) / np.sqrt(var + LN_EPS) * ln_g + ln_b
    return out.astype(np.float32)


def kernel(**inputs):
    global LAST_RESULTS
    query = np.ascontiguousarray(np.asarray(inputs["query"], dtype=np.float32))
    mem = np.ascontiguousarray(
        np.asarray(inputs["retrieved_memories"], dtype=np.float32)
    )
    sims = np.ascontiguousarray(np.asarray(inputs["similarities"], dtype=np.float32))
    mask = np.asarray(inputs["mask"])
    Wq = np.asarray(inputs["Wq"], dtype=np.float64)
    Wk = np.asarray(inputs["Wk"], dtype=np.float64)
    Wv = np.asarray(inputs["Wv"], dtype=np.float64)
    Wo = np.asarray(inputs["Wo"], dtype=np.float64)
    Wg = np.asarray(inputs["Wg"], dtype=np.float64)

    # The device kernel folds all-zero biases / identity LN affine away.
    nontrivial = (
        any(np.any(np.asarray(inputs[n])) for n in ("bq", "bk", "bv", "bo", "bg"))
        or np.any(np.asarray(inputs["ln_b"]))
        or np.any(np.asarray(inputs["ln_g"]) != 1.0)
    )
    if nontrivial or query.shape != (B, D):
        return _numpy_fallback(
            query, mem, sims, mask, Wq=Wq, bq=np.asarray(inputs["bq"]),
            Wk=Wk, bk=np.asarray(inputs["bk"]), Wv=Wv, bv=np.asarray(inputs["bv"]),
            Wo=Wo, bo=np.asarray(inputs["bo"]), Wg=Wg, bg=np.asarray(inputs["bg"]),
            ln_g=np.asarray(inputs["ln_g"]), ln_b=np.asarray(inputs["ln_b"]),
        )

    import ml_dtypes
    bf = ml_dtypes.bfloat16
    wqk = np.ascontiguousarray(((Wq @ Wk.T) * (float(D) ** -0.5)).astype(bf))
    wvo64 = Wv @ Wo
    wvo = np.ascontiguousarray(wvo64.astype(bf))
    g1 = Wg[:D, 0]
    g2 = wvo64 @ Wg[D:, 0]
    gv = np.ascontiguousarray(np.stack([g1, g2], axis=1).astype(bf))
    ident = np.eye(P, dtype=bf)
    identr = np.eye(P, dtype=np.float32)

    if "nc" not in _CACHE:
        _CACHE["nc"] = _build()
    nc = _CACHE["nc"]

    qm = np.concatenate([query, mem.reshape(B, K * D)], axis=1)
    mask_u8 = np.ascontiguousarray(mask.astype(np.uint8))
    in_maps = []
    for c in range(N_CORES):
        sl = slice(c * ROWS, (c + 1) * ROWS)
        in_maps.append({
            "qm": qm[sl], "sims": sims[sl], "mask": mask_u8[sl],
            "wqk": wqk, "wvo": wvo, "gv": gv, "ident": ident, "identr": identr,
        })

    from concourse.bass_utils import run_bass_kernel_spmd

    res = run_bass_kernel_spmd(nc, in_maps, list(range(N_CORES)), trace=TRACE)
    LAST_RESULTS = res
    return np.concatenate([res.results[c]["o"] for c in range(N_CORES)], axis=0)



# revision 9
# speedup vs baseline: 1.6418x; 1.6418x over previous
"""Memory-augmented attention kernel for Trainium2 (Bass/Tile), 8-core data parallel.

Reference computation (per row b of B=32768, D=512, K=5):
    q' = query@Wq + bq
    k  = mem@Wk + bk ; v = mem@Wv + bv
    scores = (q'.k_j)/sqrt(D) masked-softmax -> w
    mem_out = (sum_j w_j v_j)@Wo + bo
    gate = sigmoid([query, mem_out]@Wg + bg); conf = sigmoid(max_sim - 0.7)
    out = LN(query + gate*conf*mem_out) * ln_g + ln_b

Algebraic refactoring (all biases zero / identity LN affine in this problem;
a numpy fallback covers the general case):
    scores_bk = m_bk . (query_b @ (Wq Wk^T / sqrt(D)))
    mem_b     = (sum_k w_bk m_bk) @ (Wv Wo)
    gate_b    = 1/(1+exp(-(q.g1 + rsum * mcomb.(WvWo g2))))

v2 design (vs the f32 baseline):
  - All HBM I/O in bf16 (q, m, host-pretransposed qT, output) -> ~2x less DMA.
  - Softmax without max-subtraction: scores are O(1); mask penalty -60.
  - mcomb computed TRANSPOSED directly on PE: matmul(lhsT=m_k chunk,
    rhs=diag(w_k)) accumulated over k gives mcT = (sum_k w_k m_k)^T without
    any separate transpose pass.
  - qT supplied by the host in tile-transposed layout -> no PE transposes at all.
  - -g1 / -(Wvo g2) folded as a 513th column of the weight mats -> gate dots
    ride along the big matmuls with the same stationary weights.
  - Per-engine per-tile budget (est): PE ~3.5us, DVE ~3.0us, ACT ~2.7us, GP low.

This container's walrus build only encodes one sync-wait per instruction;
see _install_tile_patches.
"""

import numpy as np

B, D, K = 32768, 512, 5
N_CORES = 8
ROWS = B // N_CORES        # rows per core
P = 128                    # partitions
NT_FULL = ROWS // P        # tiles per core (32)
NCH = D // P               # 128-contraction chunks (4)
SCALE = float(D) ** -0.5
PEN = 60.0                 # mask penalty (scores are O(1), exp(-55) == 0)
LN_EPS = 1e-5
SIM_THRESH = 0.7

_CACHE = {}

TRACE = False              # set by test harness to collect a HW profile
LAST_RESULTS = None        # BassKernelResults of the last run (for profiling)


def _install_tile_patches():
    """Work around two walrus limitations in this container:
    - instructions accept very few sync-wait slots: split the kernel-tail
      drain (which Tile loads with one wait per outstanding semaphore) into
      a chain of single-wait drains;
    - EVENT_SEMAPHORE_RANGE_CLEAR is not encodable: skip the on-device sem
      clear (each kernel() call executes a freshly loaded NEFF) while keeping
      the allocator bookkeeping.
    """
    import concourse.tile as tile
    from concourse.vector_clock import ScopedClock

    if getattr(tile.TileContext._drain_and_barrier, "_patched", False):
        return

    def patched(self, tick_clock, wait_clock):
        import bass_rust

        nc = self.nc
        drain_inst = nc.sync.drain()
        wait_clock.add_sem_waits(
            drain_inst.ins, ScopedClock({None: tick_clock.global_clock})
        )
        si = drain_inst.ins.sync_info
        waits = list(si.on_wait) if si is not None and si.on_wait else []
        if len(waits) > 1:
            drain_inst.ins.sync_info = bass_rust.SyncInfo(
                on_wait=waits[:1], on_update=list(si.on_update or [])
            )
            for w in waits[1:]:
                d2 = nc.sync.drain()
                d2.ins.sync_info = bass_rust.SyncInfo(on_wait=[w], on_update=[])
        nc.all_engine_barrier()
        assert self.sems is not None
        popped = nc._tile_sem_poison_stack.pop()
        assert popped is self._sem_poison
        sems = list(self.sems.allocated().values())
        sem_nums = [s.num for s in sems]
        nc._state.prepend_free_semaphores(sem_nums)
        for poison_set in nc._tile_sem_poison_stack:
            poison_set.update(sem_nums)
        nc.all_engine_barrier()

    patched._patched = True
    tile.TileContext._drain_and_barrier = patched

    # This walrus build accepts at most one sync-wait per instruction:
    # at commit time, peel off extra waits onto single-wait nop/drain
    # instructions inserted just before the owner.
    _orig_commit = tile.TileContext._commit_instruction

    def commit_patched(self, inst, lazy_reg_writes=True):
        import bass_rust
        from concourse import mybir

        si = inst.sync_info
        if si is not None and si.on_wait and len(si.on_wait) > 1:
            waits = list(si.on_wait)
            inst.sync_info = bass_rust.SyncInfo(
                on_wait=waits[-1:], on_update=list(si.on_update or [])
            )
            for w in waits[:-1]:
                eng = self.nc.engines[inst.engine]
                if not hasattr(eng, "engine_nop"):
                    nop = mybir.InstDrain(
                        name=self.nc.get_next_instruction_name(), ins=[], outs=[]
                    )
                    nop.engine = inst.engine
                else:
                    # sequencer-only ENGINE_NOP: carries the wait without
                    # flushing the compute pipeline the way a drain does
                    nop = eng.engine_nop().ins
                nop.sync_info = bass_rust.SyncInfo(on_wait=[w], on_update=[])
                self._add_instruction(nop)
        return _orig_commit(self, inst, lazy_reg_writes)

    tile.TileContext._commit_instruction = commit_patched


def _build(ntiles=NT_FULL):
    import concourse.bass as bass
    import concourse.tile as tile
    from concourse import mybir

    _install_tile_patches()

    f32 = mybir.dt.float32
    bf16 = mybir.dt.bfloat16
    u8 = mybir.dt.uint8
    AF = mybir.ActivationFunctionType
    OP = mybir.AluOpType

    rows = ntiles * P
    rD = 1.0 / float(D)

    nc = bass.Bass()
    m_d = nc.declare_dram_parameter("m", [rows, K * D], bf16, isOutput=False)
    q_d = nc.declare_dram_parameter("q", [rows, D], bf16, isOutput=False)
    qt_d = nc.declare_dram_parameter("qt", [rows, D], bf16, isOutput=False)
    sims_d = nc.declare_dram_parameter("sims", [rows, K], f32, isOutput=False)
    mask_d = nc.declare_dram_parameter("mask", [rows, K], u8, isOutput=False)
    wqk_d = nc.declare_dram_parameter("wqk", [D, D + 1], bf16, isOutput=False)
    wvo_d = nc.declare_dram_parameter("wvo", [D, D + 1], bf16, isOutput=False)
    id_d = nc.declare_dram_parameter("ident", [P, P], bf16, isOutput=False)
    o_d = nc.declare_dram_parameter("o", [rows, D], bf16, isOutput=True)

    m_t = m_d.rearrange("(t p) d -> t p d", p=P)
    q_t = q_d.rearrange("(t p) d -> t p d", p=P)
    qt_t = qt_d.rearrange("(t p) d -> t p d", p=P)
    o_t = o_d.rearrange("(t p) d -> t p d", p=P)

    with tile.TileContext(nc) as tc:
        with (
            tc.tile_pool(name="consts", bufs=1) as consts,
            tc.tile_pool(name="mload", bufs=6) as mload,
            tc.tile_pool(name="qload", bufs=7) as qload,
            tc.tile_pool(name="qtload", bufs=3) as qtload,
            tc.tile_pool(name="work", bufs=4) as work,
            tc.tile_pool(name="smalls", bufs=7) as smalls,
            tc.tile_pool(name="ptmp", bufs=3, space="PSUM") as ptmp,
            tc.tile_pool(name="pmem", bufs=2, space="PSUM") as pmem,
            tc.tile_pool(name="pqg", bufs=1, space="PSUM") as pqg,
            tc.tile_pool(name="pmg", bufs=1, space="PSUM") as pmg,
        ):
            # ---- constants, loaded once ----
            wqk_sb = consts.tile([P, NCH, D + 1], bf16)
            nc.sync.dma_start(out=wqk_sb, in_=wqk_d.rearrange("(c p) e -> p c e", p=P))
            wvo_sb = consts.tile([P, NCH, D + 1], bf16)
            nc.sync.dma_start(out=wvo_sb, in_=wvo_d.rearrange("(c p) e -> p c e", p=P))
            identb = consts.tile([P, P], bf16)
            nc.sync.dma_start(out=identb, in_=id_d[:, :])

            sims_all = consts.tile([P, ntiles, K], f32)
            nc.sync.dma_start(
                out=sims_all, in_=sims_d.rearrange("(t p) k -> p t k", p=P)
            )
            mask_all = consts.tile([P, ntiles, K], u8)
            nc.sync.dma_start(
                out=mask_all, in_=mask_d.rearrange("(t p) k -> p t k", p=P)
            )

            thresh = consts.tile([P, 1], f32)
            nc.vector.memset(thresh, SIM_THRESH)
            epsc = consts.tile([P, 1], f32)
            nc.vector.memset(epsc, LN_EPS)
            rdc = consts.tile([P, 1], f32)
            nc.vector.memset(rdc, rD)

            # conf[b, t] = sigmoid(max_k sims - th) = 1/(1+exp(th - max))
            simmax = consts.tile([P, ntiles], f32)
            nc.vector.reduce_max(out=simmax, in_=sims_all, axis=mybir.AxisListType.X)
            confe = consts.tile([P, ntiles], f32)
            nc.scalar.activation(
                out=confe, in_=simmax, func=AF.Exp, bias=thresh, scale=-1.0
            )
            confe1 = consts.tile([P, ntiles], f32)
            nc.vector.tensor_scalar(
                out=confe1, in0=confe, scalar1=1.0, scalar2=None, op0=OP.add
            )
            conf_all = consts.tile([P, ntiles], f32)
            nc.vector.reciprocal(out=conf_all, in_=confe1)

            # pen[b, t, k] = 0 if valid else -PEN
            m01 = consts.tile([P, ntiles, K], f32)
            nc.vector.tensor_copy(out=m01, in_=mask_all)
            pen_all = consts.tile([P, ntiles, K], f32)
            nc.vector.tensor_scalar(
                out=pen_all, in0=m01, scalar1=1.0, scalar2=PEN,
                op0=OP.subtract, op1=OP.mult,
            )

            # Per-tile live state; 5-stage software pipeline (lag 4).
            st = {}

            def dma_in(t):
                s = st.setdefault(t, {})
                mt = mload.tile([P, K * D], bf16, tag="m", name="mtile")
                nc.sync.dma_start(out=mt, in_=m_t[t])
                qt_ = qload.tile([P, D], bf16, tag="q", name="qtile")
                nc.sync.dma_start(out=qt_, in_=q_t[t])
                qtt = qtload.tile([P, D], bf16, tag="qt", name="qttile")
                nc.sync.dma_start(out=qtt, in_=qt_t[t])
                s["m"] = mt
                s["q"] = qt_
                s["qT"] = qtt

            def stage_a(t):
                # t' = q@Wqk (row-major, via host-transposed qT) ; nqdot = -q.g1
                s = st[t]
                psum_t = ptmp.tile([P, D], f32, tag="ptmp", name="psum_t")
                psum_qg = pqg.tile([P, 1], f32, tag="pqg", name="psum_qg")
                for c in range(NCH):
                    sl = slice(c * P, (c + 1) * P)
                    nc.tensor.matmul(
                        psum_t,
                        lhsT=s["qT"][:, sl],
                        rhs=wqk_sb[:, c, 0:D],
                        start=(c == 0), stop=(c == NCH - 1),
                    )
                    nc.tensor.matmul(
                        psum_qg,
                        lhsT=s["qT"][:, sl],
                        rhs=wqk_sb[:, c, D:D + 1],
                        start=(c == 0), stop=(c == NCH - 1),
                    )
                tb = work.tile([P, D], bf16, tag="t_bf", name="t_bf")
                nc.scalar.copy(out=tb, in_=psum_t)
                s["t_bf"] = tb
                nq = smalls.tile([P, 1], f32, tag="nqdot", name="nqdot")
                nc.scalar.copy(out=nq, in_=psum_qg)
                s["nqdot"] = nq

            def stage_b(t):
                # raw_k = m_k . t'  (5x STT with accumulate) ; scores = raw + pen
                s = st[t]
                raw = smalls.tile([P, K], f32, tag="raw", name="raw")
                scratch = work.tile([P, D], bf16, tag="scratch", name="scratch")
                for k in range(K):
                    nc.vector.scalar_tensor_tensor(
                        out=scratch,
                        in0=s["m"][:, k * D:(k + 1) * D],
                        scalar=1.0,
                        in1=s["t_bf"],
                        op0=OP.mult, op1=OP.mult,
                        accum_out=raw[:, k:k + 1],
                    )
                sc = smalls.tile([P, K], f32, tag="scores", name="scores")
                nc.gpsimd.tensor_tensor(
                    out=sc, in0=raw, in1=pen_all[:, t, :], op=OP.add
                )
                s["scores"] = sc

            def stage_c(t):
                # w = exp(scores) (unnormalized); rsum = 1/sum(w); diag(w_k) tiles
                s = st[t]
                w = smalls.tile([P, K], f32, tag="w", name="wtile")
                sumexp = smalls.tile([P, 1], f32, tag="sumexp", name="sumexp")
                nc.scalar.activation(
                    out=w, in_=s["scores"], func=AF.Exp, accum_out=sumexp
                )
                rsum = smalls.tile([P, 1], f32, tag="rsum", name="rsum")
                nc.vector.reciprocal(out=rsum, in_=sumexp)
                s["rsum"] = rsum
                dk = work.tile([P, K * P], bf16, tag="diag", name="diag")
                for k in range(K):
                    nc.vector.tensor_scalar(
                        out=dk[:, k * P:(k + 1) * P], in0=identb,
                        scalar1=w[:, k:k + 1], scalar2=None, op0=OP.mult,
                    )
                s["dk"] = dk

            def stage_d(t):
                # mcT = (sum_k w_k m_k)^T via matmul(lhsT=m chunk, rhs=diag(w_k));
                # mem = mcomb@Wvo ; nmdot = -mcomb.g2'
                s = st[t]
                psum_mct = ptmp.tile([P, D], f32, tag="ptmp", name="psum_mct")
                for c in range(NCH):
                    sl = slice(c * P, (c + 1) * P)
                    for k in range(K):
                        nc.tensor.matmul(
                            psum_mct[:, sl],
                            lhsT=s["m"][:, k * D + c * P: k * D + (c + 1) * P],
                            rhs=s["dk"][:, k * P:(k + 1) * P],
                            start=(k == 0), stop=(k == K - 1),
                        )
                mct = work.tile([P, D], bf16, tag="mct", name="mct")
                nc.scalar.copy(out=mct, in_=psum_mct)

                pm = pmem.tile([P, D], f32, tag="pmem", name="psum_mem")
                pmgt = pmg.tile([P, 1], f32, tag="pmg", name="psum_mg")
                for c in range(NCH):
                    sl = slice(c * P, (c + 1) * P)
                    nc.tensor.matmul(
                        pm,
                        lhsT=mct[:, sl],
                        rhs=wvo_sb[:, c, 0:D],
                        start=(c == 0), stop=(c == NCH - 1),
                    )
                    nc.tensor.matmul(
                        pmgt,
                        lhsT=mct[:, sl],
                        rhs=wvo_sb[:, c, D:D + 1],
                        start=(c == 0), stop=(c == NCH - 1),
                    )
                # mg PSUM bank is single-buffered: free it immediately via a
                # small SBUF copy; ge reads the SBUF version next stage.
                nmd = smalls.tile([P, 1], f32, tag="nmdot", name="nmdot")
                nc.scalar.copy(out=nmd, in_=pmgt)
                s["pmem"] = pm
                s["nmdot"] = nmd

            def stage_e(t):
                # gate/conf scale, residual add, layernorm, store
                s = st.pop(t)
                ge = smalls.tile([P, 1], f32, tag="ge", name="ge")
                nc.scalar.activation(
                    out=ge, in_=s["nmdot"], func=AF.Exp,
                    bias=s["nqdot"], scale=s["rsum"],
                )
                gp1 = smalls.tile([P, 1], f32, tag="gp1", name="gp1")
                nc.vector.tensor_scalar(
                    out=gp1, in0=ge, scalar1=1.0, scalar2=None, op0=OP.add
                )
                rgp = smalls.tile([P, 1], f32, tag="rgp", name="rgp")
                nc.vector.reciprocal(out=rgp, in_=gp1)
                s_sb = smalls.tile([P, 1], f32, tag="s", name="s_sb")
                nc.vector.tensor_scalar(
                    out=s_sb, in0=rgp, scalar1=conf_all[:, t:t + 1],
                    scalar2=s["rsum"], op0=OP.mult, op1=OP.mult,
                )

                out_pre = work.tile([P, D], bf16, tag="out_pre", name="out_pre")
                rowsum = smalls.tile([P, 1], f32, tag="rowsum", name="rowsum")
                nc.vector.scalar_tensor_tensor(
                    out=out_pre, in0=s["pmem"], scalar=s_sb, in1=s["q"],
                    op0=OP.mult, op1=OP.add, accum_out=rowsum,
                )

                sqscr = work.tile([P, D], bf16, tag="sqscr", name="sqscr")
                sumsq = smalls.tile([P, 1], f32, tag="sumsq", name="sumsq")
                nc.scalar.activation(
                    out=sqscr, in_=out_pre, func=AF.Square, accum_out=sumsq
                )
                mu = smalls.tile([P, 1], f32, tag="mu", name="mu")
                nc.gpsimd.tensor_tensor(out=mu, in0=rowsum, in1=rdc, op=OP.mult)
                mu2 = smalls.tile([P, 1], f32, tag="mu2", name="mu2")
                nc.gpsimd.tensor_tensor(out=mu2, in0=mu, in1=mu, op=OP.mult)
                sqn = smalls.tile([P, 1], f32, tag="sqn", name="sqn")
                nc.gpsimd.tensor_tensor(out=sqn, in0=sumsq, in1=rdc, op=OP.mult)
                varc = smalls.tile([P, 1], f32, tag="varc", name="varc")
                nc.gpsimd.tensor_tensor(out=varc, in0=sqn, in1=mu2, op=OP.subtract)
                lnv = smalls.tile([P, 1], f32, tag="lnv", name="lnv")
                nc.scalar.activation(
                    out=lnv, in_=varc, func=AF.Ln, bias=epsc, scale=1.0
                )
                rstd = smalls.tile([P, 1], f32, tag="rstd", name="rstd")
                nc.scalar.activation(out=rstd, in_=lnv, func=AF.Exp, scale=-0.5)
                mr = smalls.tile([P, 1], f32, tag="mr", name="mr")
                nc.gpsimd.tensor_tensor(out=mr, in0=mu, in1=rstd, op=OP.mult)
                outf = work.tile([P, D], bf16, tag="outf", name="outf")
                nc.vector.tensor_scalar(
                    out=outf, in0=out_pre, scalar1=rstd, scalar2=mr,
                    op0=OP.mult, op1=OP.subtract,
                )
                nc.gpsimd.dma_start(out=o_t[t], in_=outf)

            dma_in(0)
            for i in range(ntiles + 4):
                if i + 1 < ntiles:
                    dma_in(i + 1)
                if i < ntiles:
                    stage_a(i)
                if 0 <= i - 4:
                    stage_e(i - 4)
                if 0 <= i - 3 <= ntiles - 1:
                    stage_d(i - 3)
                if 0 <= i - 2 <= ntiles - 1:
                    stage_c(i - 2)
                if 0 <= i - 1 <= ntiles - 1:
                    stage_b(i - 1)

    return nc


def _numpy_fallback(query, retrieved_memories, similarities, mask,
                    Wq, bq, Wk, bk, Wv, bv, Wo, bo, Wg, bg, ln_g, ln_b):
    x = query.astype(np.float64)
    m = retrieved_memories.astype(np.float64)
    q = x @ Wq + bq
    k = np.einsum("bkd,de->bke", m, Wk.astype(np.float64)) + bk
    v = np.einsum("bkd,de->bke", m, Wv.astype(np.float64)) + bv
    scores = np.einsum("bd,bkd->bk", q, k) * (D ** -0.5)
    scores = np.where(mask, scores, -np.inf)
    sm = scores - scores.max(-1, keepdims=True)
    w = np.exp(sm)
    w /= w.sum(-1, keepdims=True)
    w = np.where(mask, w, 0.0)
    mem = np.einsum("bk,bkd->bd", w, v) @ Wo + bo
    gate = 1 / (1 + np.exp(-(np.concatenate([x, mem], -1) @ Wg + bg)))
    conf = 1 / (1 + np.exp(-(similarities.max(-1, keepdims=True) - SIM_THRESH)))
    out = x + (gate * conf) * mem
    mu = out.mean(-1, keepdims=True)
    var = ((out - mu) ** 2).mean(-1, keepdims=True)
    out = (out - mu) / np.sqrt(var + LN_EPS) * ln_g + ln_b
    return out.astype(np.float32)


def kernel(**inputs):
    global LAST_RESULTS
    query = np.ascontiguousarray(np.asarray(inputs["query"], dtype=np.float32))
    mem = np.ascontiguousarray(
        np.asarray(inputs["retrieved_memories"], dtype=np.float32)
    )
    sims = np.ascontiguousarray(np.asarray(inputs["similarities"], dtype=np.float32))
    mask = np.asarray(inputs["mask"])
    Wq = np.asarray(inputs["Wq"], dtype=np.float64)
    Wk = np.asarray(inputs["Wk"], dtype=np.float64)
    Wv = np.asarray(inputs["Wv"], dtype=np.float64)
    Wo = np.asarray(inputs["Wo"], dtype=np.float64)
    Wg = np.asarray(inputs["Wg"], dtype=np.float64)

    # The device kernel folds all-zero biases / identity LN affine away.
    nontrivial = (
        any(np.any(np.asarray(inputs[n])) for n in ("bq", "bk", "bv", "bo", "bg"))
        or np.any(np.asarray(inputs["ln_b"]))
        or np.any(np.asarray(inputs["ln_g"]) != 1.0)
    )
    if nontrivial or query.shape != (B, D):
        return _numpy_fallback(
            query, mem, sims, mask, Wq=Wq, bq=np.asarray(inputs["bq"]),
            Wk=Wk, bk=np.asarray(inputs["bk"]), Wv=Wv, bv=np.asarray(inputs["bv"]),
            Wo=Wo, bo=np.asarray(inputs["bo"]), Wg=Wg, bg=np.asarray(inputs["bg"]),
            ln_g=np.asarray(inputs["ln_g"]), ln_b=np.asarray(inputs["ln_b"]),
        )

    import ml_dtypes
    bf = ml_dtypes.bfloat16
    wqk64 = (Wq @ Wk.T) * (float(D) ** -0.5)
    wvo64 = Wv @ Wo
    wqk_ext = np.empty((D, D + 1), dtype=bf)
    wqk_ext[:, :D] = wqk64.astype(bf)
    wqk_ext[:, D] = (-Wg[:D, 0]).astype(bf)
    wvo_ext = np.empty((D, D + 1), dtype=bf)
    wvo_ext[:, :D] = wvo64.astype(bf)
    wvo_ext[:, D] = (-(wvo64 @ Wg[D:, 0])).astype(bf)
    identb = np.eye(P, dtype=bf)

    q_bf = query.astype(bf)
    m_bf = mem.reshape(B, K * D).astype(bf)
    # Tile-transposed q: qt[t*128 + p, c*128 + r] = q[t*128 + r, c*128 + p]
    qt_bf = np.ascontiguousarray(
        q_bf.reshape(B // P, P, NCH, P).transpose(0, 3, 2, 1).reshape(B, D)
    )
    mask_u8 = np.ascontiguousarray(mask.astype(np.uint8))

    if "nc" not in _CACHE:
        _CACHE["nc"] = _build()
    nc = _CACHE["nc"]

    in_maps = []
    for c in range(N_CORES):
        sl = slice(c * ROWS, (c + 1) * ROWS)
        in_maps.append({
            "m": m_bf[sl], "q": q_bf[sl], "qt": qt_bf[sl],
            "sims": sims[sl], "mask": mask_u8[sl],
            "wqk": wqk_ext, "wvo": wvo_ext, "ident": identb,
        })

    from concourse.bass_utils import run_bass_kernel_spmd

    res = run_bass_kernel_spmd(nc, in_maps, list(range(N_CORES)), trace=TRACE)
    LAST_RESULTS = res
    return np.concatenate(
        [res.results[c]["o"].astype(np.float32) for c in range(N_CORES)], axis=0
    )


# revision 11
# speedup vs baseline: 1.7259x; 1.0512x over previous
"""Memory-augmented attention kernel for Trainium2 (Bass/Tile), 8-core data parallel.

Reference computation (per row b of B=32768, D=512, K=5):
    q' = query@Wq + bq
    k  = mem@Wk + bk ; v = mem@Wv + bv
    scores = (q'.k_j)/sqrt(D) masked-softmax -> w
    mem_out = (sum_j w_j v_j)@Wo + bo
    gate = sigmoid([query, mem_out]@Wg + bg); conf = sigmoid(max_sim - 0.7)
    out = LN(query + gate*conf*mem_out) * ln_g + ln_b

Algebraic refactoring (all biases zero / identity LN affine in this problem;
a numpy fallback covers the general case):
    scores_bk = m_bk . (query_b @ (Wq Wk^T / sqrt(D)))
    mem_b     = (sum_k w_bk m_bk) @ (Wv Wo)
    gate_b    = 1/(1+exp(-(q.g1 + rsum * mcomb.(WvWo g2))))

v2 design (vs the f32 baseline):
  - All HBM I/O in bf16 (q, m, host-pretransposed qT, output) -> ~2x less DMA.
  - Softmax without max-subtraction: scores are O(1); mask penalty -60.
  - mcomb computed TRANSPOSED directly on PE: matmul(lhsT=m_k chunk,
    rhs=diag(w_k)) accumulated over k gives mcT = (sum_k w_k m_k)^T without
    any separate transpose pass.
  - qT supplied by the host in tile-transposed layout -> no PE transposes at all.
  - -g1 / -(Wvo g2) folded as a 513th column of the weight mats -> gate dots
    ride along the big matmuls with the same stationary weights.
  - Per-engine per-tile budget (est): PE ~3.5us, DVE ~3.0us, ACT ~2.7us, GP low.

This container's walrus build only encodes one sync-wait per instruction;
see _install_tile_patches.
"""

import numpy as np

B, D, K = 32768, 512, 5
N_CORES = 8
ROWS = B // N_CORES        # rows per core
P = 128                    # partitions
NT_FULL = ROWS // P        # tiles per core (32)
NCH = D // P               # 128-contraction chunks (4)
SCALE = float(D) ** -0.5
PEN = 60.0                 # mask penalty (scores are O(1), exp(-55) == 0)
LN_EPS = 1e-5
SIM_THRESH = 0.7

_CACHE = {}

TRACE = False              # set by test harness to collect a HW profile
LAST_RESULTS = None        # BassKernelResults of the last run (for profiling)


def _install_tile_patches():
    """Work around two walrus limitations in this container:
    - instructions accept very few sync-wait slots: split the kernel-tail
      drain (which Tile loads with one wait per outstanding semaphore) into
      a chain of single-wait drains;
    - EVENT_SEMAPHORE_RANGE_CLEAR is not encodable: skip the on-device sem
      clear (each kernel() call executes a freshly loaded NEFF) while keeping
      the allocator bookkeeping.
    """
    import concourse.tile as tile
    from concourse.vector_clock import ScopedClock

    if getattr(tile.TileContext._drain_and_barrier, "_patched", False):
        return

    def patched(self, tick_clock, wait_clock):
        import bass_rust

        nc = self.nc
        drain_inst = nc.sync.drain()
        wait_clock.add_sem_waits(
            drain_inst.ins, ScopedClock({None: tick_clock.global_clock})
        )
        si = drain_inst.ins.sync_info
        waits = list(si.on_wait) if si is not None and si.on_wait else []
        if len(waits) > 1:
            drain_inst.ins.sync_info = bass_rust.SyncInfo(
                on_wait=waits[:1], on_update=list(si.on_update or [])
            )
            for w in waits[1:]:
                d2 = nc.sync.drain()
                d2.ins.sync_info = bass_rust.SyncInfo(on_wait=[w], on_update=[])
        nc.all_engine_barrier()
        assert self.sems is not None
        popped = nc._tile_sem_poison_stack.pop()
        assert popped is self._sem_poison
        sems = list(self.sems.allocated().values())
        sem_nums = [s.num for s in sems]
        nc._state.prepend_free_semaphores(sem_nums)
        for poison_set in nc._tile_sem_poison_stack:
            poison_set.update(sem_nums)
        nc.all_engine_barrier()

    patched._patched = True
    tile.TileContext._drain_and_barrier = patched

    # This walrus build accepts at most one sync-wait per instruction:
    # at commit time, peel off extra waits onto single-wait nop/drain
    # instructions inserted just before the owner.
    _orig_commit = tile.TileContext._commit_instruction

    def commit_patched(self, inst, lazy_reg_writes=True):
        import bass_rust
        from concourse import mybir

        si = inst.sync_info
        if si is not None and si.on_wait and len(si.on_wait) > 1:
            waits = list(si.on_wait)
            inst.sync_info = bass_rust.SyncInfo(
                on_wait=waits[-1:], on_update=list(si.on_update or [])
            )
            for w in waits[:-1]:
                eng = self.nc.engines[inst.engine]
                if not hasattr(eng, "engine_nop"):
                    nop = mybir.InstDrain(
                        name=self.nc.get_next_instruction_name(), ins=[], outs=[]
                    )
                    nop.engine = inst.engine
                else:
                    # sequencer-only ENGINE_NOP: carries the wait without
                    # flushing the compute pipeline the way a drain does
                    nop = eng.engine_nop().ins
                nop.sync_info = bass_rust.SyncInfo(on_wait=[w], on_update=[])
                self._add_instruction(nop)
        return _orig_commit(self, inst, lazy_reg_writes)

    tile.TileContext._commit_instruction = commit_patched


def _build(ntiles=NT_FULL):
    import concourse.bass as bass
    import concourse.tile as tile
    from concourse import mybir

    _install_tile_patches()

    f32 = mybir.dt.float32
    bf16 = mybir.dt.bfloat16
    u8 = mybir.dt.uint8
    AF = mybir.ActivationFunctionType
    OP = mybir.AluOpType

    rows = ntiles * P
    rD = 1.0 / float(D)

    nc = bass.Bass()
    mqt_d = nc.declare_dram_parameter(
        "mqt", [rows, (K + 2) * D], bf16, isOutput=False
    )
    sims_d = nc.declare_dram_parameter("sims", [rows, K], f32, isOutput=False)
    mask_d = nc.declare_dram_parameter("mask", [rows, K], u8, isOutput=False)
    wqk_d = nc.declare_dram_parameter("wqk", [D, D + 1], bf16, isOutput=False)
    wvo_d = nc.declare_dram_parameter("wvo", [D, D + 1], bf16, isOutput=False)
    id_d = nc.declare_dram_parameter("ident", [P, P], bf16, isOutput=False)
    o_d = nc.declare_dram_parameter("o", [rows, D], bf16, isOutput=True)

    mqt_t = mqt_d.rearrange("(t p) d -> t p d", p=P)
    o_t = o_d.rearrange("(t p) d -> t p d", p=P)

    with tile.TileContext(nc) as tc:
        with (
            tc.tile_pool(name="consts", bufs=1) as consts,
            tc.tile_pool(name="mload", bufs=6) as mload,
            tc.tile_pool(name="work", bufs=4) as work,
            tc.tile_pool(name="smalls", bufs=7) as smalls,
            tc.tile_pool(name="ptmp", bufs=3, space="PSUM") as ptmp,
            tc.tile_pool(name="pmem", bufs=2, space="PSUM") as pmem,
            tc.tile_pool(name="pqg", bufs=1, space="PSUM") as pqg,
            tc.tile_pool(name="pmg", bufs=1, space="PSUM") as pmg,
        ):
            # ---- constants, loaded once ----
            wqk_sb = consts.tile([P, NCH, D + 1], bf16)
            nc.sync.dma_start(out=wqk_sb, in_=wqk_d.rearrange("(c p) e -> p c e", p=P))
            wvo_sb = consts.tile([P, NCH, D + 1], bf16)
            nc.sync.dma_start(out=wvo_sb, in_=wvo_d.rearrange("(c p) e -> p c e", p=P))
            identb = consts.tile([P, P], bf16)
            nc.sync.dma_start(out=identb, in_=id_d[:, :])

            sims_all = consts.tile([P, ntiles, K], f32)
            nc.sync.dma_start(
                out=sims_all, in_=sims_d.rearrange("(t p) k -> p t k", p=P)
            )
            mask_all = consts.tile([P, ntiles, K], u8)
            nc.sync.dma_start(
                out=mask_all, in_=mask_d.rearrange("(t p) k -> p t k", p=P)
            )

            thresh = consts.tile([P, 1], f32)
            nc.vector.memset(thresh, SIM_THRESH)
            epsc = consts.tile([P, 1], f32)
            nc.vector.memset(epsc, LN_EPS)
            onec = consts.tile([P, 1], f32)
            nc.vector.memset(onec, 1.0)

            # conf[b, t] = sigmoid(max_k sims - th) = 1/(1+exp(th - max))
            simmax = consts.tile([P, ntiles], f32)
            nc.vector.reduce_max(out=simmax, in_=sims_all, axis=mybir.AxisListType.X)
            confe = consts.tile([P, ntiles], f32)
            nc.scalar.activation(
                out=confe, in_=simmax, func=AF.Exp, bias=thresh, scale=-1.0
            )
            confe1 = consts.tile([P, ntiles], f32)
            nc.vector.tensor_scalar(
                out=confe1, in0=confe, scalar1=1.0, scalar2=None, op0=OP.add
            )
            conf_all = consts.tile([P, ntiles], f32)
            nc.vector.reciprocal(out=conf_all, in_=confe1)

            # pen[b, t, k] = 0 if valid else -PEN
            m01 = consts.tile([P, ntiles, K], f32)
            nc.vector.tensor_copy(out=m01, in_=mask_all)
            pen_all = consts.tile([P, ntiles, K], f32)
            nc.vector.tensor_scalar(
                out=pen_all, in0=m01, scalar1=1.0, scalar2=PEN,
                op0=OP.subtract, op1=OP.mult,
            )

            # Per-tile live state; 5-stage software pipeline (lag 4).
            st = {}

            def dma_in(t):
                s = st.setdefault(t, {})
                mqt = mload.tile([P, (K + 2) * D], bf16, tag="mqt", name="mqt")
                nc.sync.dma_start(out=mqt, in_=mqt_t[t])
                s["m"] = mqt[:, 0:K * D]
                s["q"] = mqt[:, K * D:(K + 1) * D]
                s["qT"] = mqt[:, (K + 1) * D:(K + 2) * D]

            def stage_a(t):
                # t' = q@Wqk (row-major, via host-transposed qT) ; nqdot = -q.g1
                s = st[t]
                psum_t = ptmp.tile([P, D], f32, tag="ptmp", name="psum_t")
                psum_qg = pqg.tile([P, 1], f32, tag="pqg", name="psum_qg")
                for c in range(NCH):
                    sl = slice(c * P, (c + 1) * P)
                    nc.tensor.matmul(
                        psum_t,
                        lhsT=s["qT"][:, sl],
                        rhs=wqk_sb[:, c, 0:D],
                        start=(c == 0), stop=(c == NCH - 1),
                    )
                    nc.tensor.matmul(
                        psum_qg,
                        lhsT=s["qT"][:, sl],
                        rhs=wqk_sb[:, c, D:D + 1],
                        start=(c == 0), stop=(c == NCH - 1),
                    )
                tb = work.tile([P, D], bf16, tag="t_bf", name="t_bf")
                nc.scalar.copy(out=tb, in_=psum_t)
                s["t_bf"] = tb
                nq = smalls.tile([P, 1], f32, tag="nqdot", name="nqdot")
                nc.scalar.copy(out=nq, in_=psum_qg)
                s["nqdot"] = nq

            def stage_b(t):
                # raw_k = m_k . t'  (5x STT with accumulate) ; scores = raw + pen
                s = st[t]
                raw = smalls.tile([P, K], f32, tag="raw", name="raw")
                scratch = work.tile([P, D], bf16, tag="scratch", name="scratch")
                for k in range(K):
                    nc.vector.scalar_tensor_tensor(
                        out=scratch,
                        in0=s["m"][:, k * D:(k + 1) * D],
                        scalar=1.0,
                        in1=s["t_bf"],
                        op0=OP.mult, op1=OP.mult,
                        accum_out=raw[:, k:k + 1],
                    )
                sc = smalls.tile([P, K], f32, tag="scores", name="scores")
                nc.gpsimd.tensor_tensor(
                    out=sc, in0=raw, in1=pen_all[:, t, :], op=OP.add
                )
                s["scores"] = sc

            def stage_c(t):
                # w = exp(scores) (unnormalized); rsum = 1/sum(w); diag(w_k) tiles
                s = st[t]
                w = smalls.tile([P, K], f32, tag="w", name="wtile")
                sumexp = smalls.tile([P, 1], f32, tag="sumexp", name="sumexp")
                nc.scalar.activation(
                    out=w, in_=s["scores"], func=AF.Exp, accum_out=sumexp
                )
                rsum = smalls.tile([P, 1], f32, tag="rsum", name="rsum")
                nc.vector.reciprocal(out=rsum, in_=sumexp)
                s["rsum"] = rsum
                dk = work.tile([P, K, P], bf16, tag="diag", name="diag")
                nc.vector.tensor_tensor(
                    out=dk[:, :, :],
                    in0=identb[:, :].rearrange(
                        "p (o j) -> p o j", o=1).broadcast_to([P, K, P]),
                    in1=w[:, :].rearrange(
                        "p (k o) -> p k o", o=1).broadcast_to([P, K, P]),
                    op=OP.mult,
                )
                s["dk"] = dk

            def stage_d(t):
                # mcT = (sum_k w_k m_k)^T via matmul(lhsT=m chunk, rhs=diag(w_k));
                # mem = mcomb@Wvo ; nmdot = -mcomb.g2'
                s = st[t]
                psum_mct = ptmp.tile([P, D], f32, tag="ptmp", name="psum_mct")
                for c in range(NCH):
                    sl = slice(c * P, (c + 1) * P)
                    for k in range(K):
                        nc.tensor.matmul(
                            psum_mct[:, sl],
                            lhsT=s["m"][:, k * D + c * P: k * D + (c + 1) * P],
                            rhs=s["dk"][:, k, :],
                            start=(k == 0), stop=(k == K - 1),
                        )
                mct = work.tile([P, D], bf16, tag="mct", name="mct")
                nc.scalar.copy(out=mct, in_=psum_mct)

                pm = pmem.tile([P, D], f32, tag="pmem", name="psum_mem")
                pmgt = pmg.tile([P, 1], f32, tag="pmg", name="psum_mg")
                for c in range(NCH):
                    sl = slice(c * P, (c + 1) * P)
                    nc.tensor.matmul(
                        pm,
                        lhsT=mct[:, sl],
                        rhs=wvo_sb[:, c, 0:D],
                        start=(c == 0), stop=(c == NCH - 1),
                    )
                    nc.tensor.matmul(
                        pmgt,
                        lhsT=mct[:, sl],
                        rhs=wvo_sb[:, c, D:D + 1],
                        start=(c == 0), stop=(c == NCH - 1),
                    )
                # mg PSUM bank is single-buffered: free it immediately via a
                # small SBUF copy; ge reads the SBUF version next stage.
                nmd = smalls.tile([P, 1], f32, tag="nmdot", name="nmdot")
                nc.scalar.copy(out=nmd, in_=pmgt)
                s["pmem"] = pm
                s["nmdot"] = nmd

            def stage_e(t):
                # gate/conf scale, residual add, layernorm, store
                s = st.pop(t)
                ge = smalls.tile([P, 1], f32, tag="ge", name="ge")
                nc.scalar.activation(
                    out=ge, in_=s["nmdot"], func=AF.Exp,
                    bias=s["nqdot"], scale=s["rsum"],
                )
                gp1 = smalls.tile([P, 1], f32, tag="gp1", name="gp1")
                nc.gpsimd.tensor_tensor(out=gp1, in0=ge, in1=onec, op=OP.add)
                rgp = smalls.tile([P, 1], f32, tag="rgp", name="rgp")
                nc.vector.reciprocal(out=rgp, in_=gp1)
                s_sb = smalls.tile([P, 1], f32, tag="s", name="s_sb")
                nc.vector.tensor_scalar(
                    out=s_sb, in0=rgp, scalar1=conf_all[:, t:t + 1],
                    scalar2=s["rsum"], op0=OP.mult, op1=OP.mult,
                )

                out_pre = work.tile([P, D], bf16, tag="out_pre", name="out_pre")
                rowsum = smalls.tile([P, 1], f32, tag="rowsum", name="rowsum")
                nc.vector.scalar_tensor_tensor(
                    out=out_pre, in0=s["pmem"], scalar=s_sb, in1=s["q"],
                    op0=OP.mult, op1=OP.add, accum_out=rowsum,
                )

                sqscr = work.tile([P, D], bf16, tag="sqscr", name="sqscr")
                sumsq = smalls.tile([P, 1], f32, tag="sumsq", name="sumsq")
                nc.scalar.activation(
                    out=sqscr, in_=out_pre, func=AF.Square, accum_out=sumsq
                )
                mu2 = smalls.tile([P, 1], f32, tag="mu2", name="mu2")
                nc.vector.tensor_scalar(
                    out=mu2, in0=rowsum, scalar1=rowsum[:, 0:1],
                    scalar2=rD * rD, op0=OP.mult, op1=OP.mult,
                )
                varc = smalls.tile([P, 1], f32, tag="varc", name="varc")
                nc.vector.tensor_scalar(
                    out=varc, in0=sumsq, scalar1=rD,
                    scalar2=mu2[:, 0:1], op0=OP.mult, op1=OP.subtract,
                )
                lnv = smalls.tile([P, 1], f32, tag="lnv", name="lnv")
                nc.scalar.activation(
                    out=lnv, in_=varc, func=AF.Ln, bias=epsc, scale=1.0
                )
                rstd = smalls.tile([P, 1], f32, tag="rstd", name="rstd")
                nc.scalar.activation(out=rstd, in_=lnv, func=AF.Exp, scale=-0.5)
                mr = smalls.tile([P, 1], f32, tag="mr", name="mr")
                nc.vector.tensor_scalar(
                    out=mr, in0=rstd, scalar1=rowsum[:, 0:1],
                    scalar2=rD, op0=OP.mult, op1=OP.mult,
                )
                outf = work.tile([P, D], bf16, tag="outf", name="outf")
                nc.vector.tensor_scalar(
                    out=outf, in0=out_pre, scalar1=rstd, scalar2=mr,
                    op0=OP.mult, op1=OP.subtract,
                )
                nc.gpsimd.dma_start(out=o_t[t], in_=outf)

            dma_in(0)
            for i in range(ntiles + 4):
                if i + 1 < ntiles:
                    dma_in(i + 1)
                if i < ntiles:
                    stage_a(i)
                if 0 <= i - 4:
                    stage_e(i - 4)
                if 0 <= i - 3 <= ntiles - 1:
                    stage_d(i - 3)
                if 0 <= i - 2 <= ntiles - 1:
                    stage_c(i - 2)
                if 0 <= i - 1 <= ntiles - 1:
                    stage_b(i - 1)

    return nc


def _numpy_fallback(query, retrieved_memories, similarities, mask,
                    Wq, bq, Wk, bk, Wv, bv, Wo, bo, Wg, bg, ln_g, ln_b):
    x = query.astype(np.float64)
    m = retrieved_memories.astype(np.float64)
    q = x @ Wq + bq
    k = np.einsum("bkd,de->bke", m, Wk.astype(np.float64)) + bk
    v = np.einsum("bkd,de->bke", m, Wv.astype(np.float64)) + bv
    scores = np.einsum("bd,bkd->bk", q, k) * (D ** -0.5)
    scores = np.where(mask, scores, -np.inf)
    sm = scores - scores.max(-1, keepdims=True)
    w = np.exp(sm)
    w /= w.sum(-1, keepdims=True)
    w = np.where(mask, w, 0.0)
    mem = np.einsum("bk,bkd->bd", w, v) @ Wo + bo
    gate = 1 / (1 + np.exp(-(np.concatenate([x, mem], -1) @ Wg + bg)))
    conf = 1 / (1 + np.exp(-(similarities.max(-1, keepdims=True) - SIM_THRESH)))
    out = x + (gate * conf) * mem
    mu = out.mean(-1, keepdims=True)
    var = ((out - mu) ** 2).mean(-1, keepdims=True)
    out = (out - mu) / np.sqrt(var + LN_EPS) * ln_g + ln_b
    return out.astype(np.float32)


def kernel(**inputs):
    global LAST_RESULTS
    query = np.ascontiguousarray(np.asarray(inputs["query"], dtype=np.float32))
    mem = np.ascontiguousarray(
        np.asarray(inputs["retrieved_memories"], dtype=np.float32)
    )
    sims = np.ascontiguousarray(np.asarray(inputs["similarities"], dtype=np.float32))
    mask = np.asarray(inputs["mask"])
    Wq = np.asarray(inputs["Wq"], dtype=np.float64)
    Wk = np.asarray(inputs["Wk"], dtype=np.float64)
    Wv = np.asarray(inputs["Wv"], dtype=np.float64)
    Wo = np.asarray(inputs["Wo"], dtype=np.float64)
    Wg = np.asarray(inputs["Wg"], dtype=np.float64)

    # The device kernel folds all-zero biases / identity LN affine away.
    nontrivial = (
        any(np.any(np.asarray(inputs[n])) for n in ("bq", "bk", "bv", "bo", "bg"))
        or np.any(np.asarray(inputs["ln_b"]))
        or np.any(np.asarray(inputs["ln_g"]) != 1.0)
    )
    if nontrivial or query.shape != (B, D):
        return _numpy_fallback(
            query, mem, sims, mask, Wq=Wq, bq=np.asarray(inputs["bq"]),
            Wk=Wk, bk=np.asarray(inputs["bk"]), Wv=Wv, bv=np.asarray(inputs["bv"]),
            Wo=Wo, bo=np.asarray(inputs["bo"]), Wg=Wg, bg=np.asarray(inputs["bg"]),
            ln_g=np.asarray(inputs["ln_g"]), ln_b=np.asarray(inputs["ln_b"]),
        )

    import ml_dtypes
    bf = ml_dtypes.bfloat16
    wqk64 = (Wq @ Wk.T) * (float(D) ** -0.5)
    wvo64 = Wv @ Wo
    wqk_ext = np.empty((D, D + 1), dtype=bf)
    wqk_ext[:, :D] = wqk64.astype(bf)
    wqk_ext[:, D] = (-Wg[:D, 0]).astype(bf)
    wvo_ext = np.empty((D, D + 1), dtype=bf)
    wvo_ext[:, :D] = wvo64.astype(bf)
    wvo_ext[:, D] = (-(wvo64 @ Wg[D:, 0])).astype(bf)
    identb = np.eye(P, dtype=bf)

    q_bf = query.astype(bf)
    mqt = np.empty((B, (K + 2) * D), dtype=bf)
    mqt[:, :K * D] = mem.reshape(B, K * D)
    mqt[:, K * D:(K + 1) * D] = q_bf
    # Tile-transposed q: qt[t*128 + p, c*128 + r] = q[t*128 + r, c*128 + p]
    mqt[:, (K + 1) * D:] = (
        q_bf.reshape(B // P, P, NCH, P).transpose(0, 3, 2, 1).reshape(B, D)
    )
    mask_u8 = np.ascontiguousarray(mask.astype(np.uint8))

    if "nc" not in _CACHE:
        _CACHE["nc"] = _build()
    nc = _CACHE["nc"]

    in_maps = []
    for c in range(N_CORES):
        sl = slice(c * ROWS, (c + 1) * ROWS)
        in_maps.append({
            "mqt": mqt[sl],
            "sims": sims[sl], "mask": mask_u8[sl],
            "wqk": wqk_ext, "wvo": wvo_ext, "ident": identb,
        })

    from concourse.bass_utils import run_bass_kernel_spmd

    res = run_bass_kernel_spmd(nc, in_maps, list(range(N_CORES)), trace=TRACE)
    LAST_RESULTS = res
    return np.concatenate(
        [res.results[c]["o"].astype(np.float32) for c in range(N_CORES)], axis=0
    )
